# revision 1
# baseline (speedup 1.0000x reference)
"""GQA sliding-window attention (RoPE + RMSNorm + tanh soft-cap) on 8 trn2 cores.

Sharding: data-parallel over batch (2) x tensor-parallel over KV heads (4).
Core c handles batch b = c//4 and kv-head g = c%4 (2 query heads each).
o_proj is row-parallel; the 4 partial [L, D] outputs per batch are summed on
the host during unsharding.

Device layout choices (all matmuls fp32r, ~tf32 precision at bf16 speed):
  - x is fed transposed (xT [D, L]) so the QKV contraction dim D sits on
    SBUF partitions for both operands.
  - q/k are produced transposed ([head_dim, L]) which is what the scores
    matmul wants; v is produced natural ([L, head_dim]) which is what the
    PV matmul wants.
  - scores are computed transposed (S^T [j, i]); softmax denominators come
    from an extra ones-weights matmul accumulated alongside PV.
  - RMSNorm partition-dim reductions use a ones-weights matmul; the result
    arrives pre-broadcast across partitions.
  - causal + sliding-window masking is done with gpsimd.affine_select on the
    ~2/3 of score tiles that straddle a mask boundary; interior tiles skip it.
"""

import numpy as np

import concourse.bass as bass
import concourse.bacc as bacc
import concourse.tile as tile
from concourse import mybir
from concourse.bass_utils import run_bass_kernel_spmd

F32 = mybir.dt.float32
F32R = mybir.dt.float32r
AF = mybir.ActivationFunctionType
OP = mybir.AluOpType

B, L, D = 2, 2048, 2048
NUM_HEADS, NUM_KV_HEADS, HEAD_DIM = 8, 4, 256
GROUPS = NUM_HEADS // NUM_KV_HEADS  # 2
ROPE_BASE, ROPE_SCALE = 10000.0, 1.0
WINDOW = 1024
SOFT_CAP = 50.0
EPS = 1e-6
P = 128
IB = 512                      # query block
NB = L // IB                  # 4 query blocks
DT = D // P                   # 16 contraction tiles
HALF = HEAD_DIM // 2          # 128 (rope half == one partition tile)
NEG_BIG = -1e38


def _jts(ib):
    """key tiles (of 128) overlapping the causal sliding window of query
    block ib ([ib*512, ib*512+511])."""
    i0 = ib * IB
    lo = max(0, (i0 - (WINDOW - 1)) // P)
    hi = (i0 + IB - 1) // P
    return list(range(lo, hi + 1))


def _build():
    nc = bacc.Bacc("TRN2", target_bir_lowering=False, debug=False, num_devices=8)

    xT = nc.dram_tensor("xT", [D, L], F32R, kind="ExternalInput")
    wq = nc.dram_tensor("wq", [D, GROUPS * HEAD_DIM], F32R, kind="ExternalInput")
    wk = nc.dram_tensor("wk", [D, HEAD_DIM], F32R, kind="ExternalInput")
    wv = nc.dram_tensor("wv", [D, HEAD_DIM], F32R, kind="ExternalInput")
    wo = nc.dram_tensor("wo", [GROUPS * HEAD_DIM, D], F32R, kind="ExternalInput")
    cosT = nc.dram_tensor("cosT", [HALF, L], F32, kind="ExternalInput")
    sinT = nc.dram_tensor("sinT", [HALF, L], F32, kind="ExternalInput")
    qsc = nc.dram_tensor("qsc", [P, 2], F32, kind="ExternalInput")
    ksc = nc.dram_tensor("ksc", [P, 2], F32, kind="ExternalInput")
    outd = nc.dram_tensor("out", [L, D], F32, kind="ExternalOutput")

    from contextlib import ExitStack
    with tile.TileContext(nc) as tc, ExitStack() as _ctx:
        with tc.tile_pool(name="konst", bufs=1) as konst, \
             tc.tile_pool(name="kv_sb", bufs=1) as kv_sb:
            ones_f = konst.tile([P, P], F32)
            nc.vector.memset(ones_f, 1.0)
            ones = konst.tile([P, P], F32R)
            nc.vector.tensor_copy(out=ones, in_=ones_f)
            nbias = konst.tile([P, 1], F32)
            nc.vector.memset(nbias, -SOFT_CAP)
            epsb = konst.tile([P, 1], F32)
            nc.vector.memset(epsb, EPS)

            kT_sb = kv_sb.tile([P, 2, L], F32R)       # k^T (roped), h-halves
            v_sb = kv_sb.tile([P, DT, HEAD_DIM], F32R)  # v natural, j-tiles
            qdramp = _ctx.enter_context(
                tc.tile_pool(name="qdramp", bufs=1, space="DRAM"))
            qdram = qdramp.tile([P, 2 * GROUPS, L], F32R)

            # ---------------- Phase A: projections + norm + rope -----------
            with tc.tile_pool(name="wA", bufs=1) as wA, \
                 tc.tile_pool(name="xA", bufs=6) as xA, \
                 tc.tile_pool(name="csA", bufs=2) as csA, \
                 tc.tile_pool(name="tmpA", bufs=2) as tmpA, \
                 tc.tile_pool(name="ropeA", bufs=2) as ropeA, \
                 tc.tile_pool(name="qstage", bufs=2) as qstage, \
                 tc.tile_pool(name="psProj", bufs=4, space="PSUM") as psProj, \
                 tc.tile_pool(name="psV", bufs=2, space="PSUM") as psV, \
                 tc.tile_pool(name="psAux", bufs=2, space="PSUM") as psAux:

                wq_s = wA.tile([P, DT, GROUPS * HEAD_DIM], F32R)
                nc.sync.dma_start(
                    out=wq_s, in_=wq.ap().rearrange("(dt dk) c -> dk dt c", dk=P))
                wk_s = wA.tile([P, DT, HEAD_DIM], F32R)
                nc.sync.dma_start(
                    out=wk_s, in_=wk.ap().rearrange("(dt dk) c -> dk dt c", dk=P))
                wv_s = wA.tile([P, DT, HEAD_DIM], F32R)
                nc.sync.dma_start(
                    out=wv_s, in_=wv.ap().rearrange("(dt dk) c -> dk dt c", dk=P))
                qsc_s = wA.tile([P, 2], F32)
                nc.sync.dma_start(out=qsc_s, in_=qsc.ap())
                ksc_s = wA.tile([P, 2], F32)
                nc.sync.dma_start(out=ksc_s, in_=ksc.ap())

                def norm_rope(ps0, ps1, sc, cosb, sinb, dst0, dst1):
                    """RMSNorm (over 256 = both psum tiles' partitions) +
                    per-channel scale + rope; writes f32r dst tiles."""
                    sq0 = tmpA.tile([P, IB], F32R, tag="sq")
                    nc.scalar.activation(out=sq0, in_=ps0, func=AF.Square)
                    sq1 = tmpA.tile([P, IB], F32R, tag="sq")
                    nc.scalar.activation(out=sq1, in_=ps1, func=AF.Square)
                    ssps = psAux.tile([P, IB], F32)
                    nc.tensor.matmul(ssps, lhsT=ones, rhs=sq0, start=True, stop=False)
                    nc.tensor.matmul(ssps, lhsT=ones, rhs=sq1, start=False, stop=True)
                    srt = tmpA.tile([P, IB], F32, tag="srt")
                    nc.scalar.activation(out=srt, in_=ssps, func=AF.Sqrt,
                                         scale=1.0 / HEAD_DIM, bias=epsb)
                    rs = tmpA.tile([P, IB], F32, tag="rs")
                    nc.vector.reciprocal(out=rs, in_=srt)
                    a0 = ropeA.tile([P, IB], F32, tag="a0")
                    nc.vector.scalar_tensor_tensor(
                        out=a0, in0=ps0, scalar=sc[:, 0:1], in1=rs,
                        op0=OP.mult, op1=OP.mult)
                    a1 = ropeA.tile([P, IB], F32, tag="a1")
                    nc.vector.scalar_tensor_tensor(
                        out=a1, in0=ps1, scalar=sc[:, 1:2], in1=rs,
                        op0=OP.mult, op1=OP.mult)
                    t0 = ropeA.tile([P, IB], F32, tag="t0")
                    nc.vector.tensor_tensor(out=t0, in0=a0, in1=cosb, op=OP.mult)
                    t1 = ropeA.tile([P, IB], F32, tag="t1")
                    nc.vector.tensor_tensor(out=t1, in0=a1, in1=sinb, op=OP.mult)
                    nc.vector.tensor_tensor(out=dst0, in0=t0, in1=t1, op=OP.subtract)
                    t2 = ropeA.tile([P, IB], F32, tag="t0")
                    nc.vector.tensor_tensor(out=t2, in0=a1, in1=cosb, op=OP.mult)
                    t3 = ropeA.tile([P, IB], F32, tag="t1")
                    nc.vector.tensor_tensor(out=t3, in0=a0, in1=sinb, op=OP.mult)
                    nc.vector.tensor_tensor(out=dst1, in0=t2, in1=t3, op=OP.add)

                for n in range(NB):
                    nsl = slice(n * IB, (n + 1) * IB)
                    xq = []
                    for qd in range(4):
                        xt_ = xA.tile([P, DT // 4, IB], F32R, tag="xh",
                                      name=f"xh_{n}_{qd}")
                        nc.sync.dma_start(
                            out=xt_,
                            in_=xT.ap()[qd * D // 4:(qd + 1) * D // 4,
                                        nsl].rearrange(
                                "(do dk) n -> dk do n", dk=P))
                        xq.append(xt_)

                    def dtile(dt):
                        return xq[dt // 4][:, dt % 4, :]

                    cosb = csA.tile([P, IB], F32, tag="cos")
                    nc.sync.dma_start(out=cosb, in_=cosT.ap()[:, nsl])
                    sinb = csA.tile([P, IB], F32, tag="sin")
                    nc.sync.dma_start(out=sinb, in_=sinT.ap()[:, nsl])

                    # k projection (2 h-half tiles)
                    kps = []
                    for ct in range(2):
                        ps = psProj.tile([P, IB], F32, tag="proj")
                        for dt in range(DT):
                            nc.tensor.matmul(
                                ps, lhsT=wk_s[:, dt, ct * P:(ct + 1) * P],
                                rhs=dtile(dt), start=dt == 0, stop=dt == DT - 1)
                        kps.append(ps)
                    norm_rope(kps[0], kps[1], ksc_s, cosb, sinb,
                              kT_sb[:, 0, nsl], kT_sb[:, 1, nsl])
                    # v projection (natural layout)
                    for st in range(4):
                        ps = psV.tile([P, HEAD_DIM], F32)
                        for dt in range(DT):
                            nc.tensor.matmul(
                                ps, lhsT=dtile(dt)[:, st * P:(st + 1) * P],
                                rhs=wv_s[:, dt, :], start=dt == 0, stop=dt == DT - 1)
                        nc.scalar.activation(out=v_sb[:, n * 4 + st, :], in_=ps, func=AF.Copy)

                    # q projection (per head)
                    for head in range(GROUPS):
                        qps = []
                        for h in range(2):
                            ct = head * 2 + h
                            ps = psProj.tile([P, IB], F32, tag="proj")
                            for dt in range(DT):
                                nc.tensor.matmul(
                                    ps, lhsT=wq_s[:, dt, ct * P:(ct + 1) * P],
                                    rhs=dtile(dt), start=dt == 0, stop=dt == DT - 1)
                            qps.append(ps)
                        qd0 = qstage.tile([P, IB], F32R, tag="qd0")
                        qd1 = qstage.tile([P, IB], F32R, tag="qd1")
                        norm_rope(qps[0], qps[1], qsc_s, cosb, sinb, qd0, qd1)
                        nc.sync.dma_start(
                            out=qdram[:, head * 2, nsl], in_=qd0)
                        nc.sync.dma_start(
                            out=qdram[:, head * 2 + 1, nsl], in_=qd1)


            # ---------------- Phase B: attention + o_proj ------------------
            with tc.tile_pool(name="qB", bufs=3) as qB, \
                 tc.tile_pool(name="wBp", bufs=1) as wBp, \
                 tc.tile_pool(name="sS", bufs=5) as sS, \
                 tc.tile_pool(name="pPT", bufs=5) as pPT, \
                 tc.tile_pool(name="oT", bufs=3) as oT, \
                 tc.tile_pool(name="dR", bufs=2) as dR, \
                 tc.tile_pool(name="osb", bufs=6) as osb, \
                 tc.tile_pool(name="psO", bufs=4, space="PSUM") as psO, \
                 tc.tile_pool(name="psS", bufs=2, space="PSUM") as psS, \
                 tc.tile_pool(name="psD", bufs=2, space="PSUM") as psD:
                psOut = psD

                wo_s = wBp.tile([P, 2 * GROUPS, D], F32R)
                nc.sync.dma_start(
                    out=wo_s, in_=wo.ap().rearrange("(ct ck) e -> ck ct e", ck=P))

                for ib in range(NB):
                    jts = _jts(ib)
                    isl = slice(ib * IB, (ib + 1) * IB)
                    qblk = qB.tile([P, 2 * GROUPS, IB], F32R)
                    nc.sync.dma_start(out=qblk, in_=qdram[:, :, isl])
                    ot_t = oT.tile([P, 2 * GROUPS, IB], F32R)
                    for head in range(GROUPS):
                        ops = [psO.tile([P, IB], F32, tag="psO", name=f"psO_{head}_{_}") for _ in range(2)]
                        dps = psD.tile([P, IB], F32, tag="psDO", name=f"dps_{head}")
                        for idx, jt in enumerate(jts):
                            # active column range [o, e) of this score tile
                            # (queries outside it are fully masked); the first
                            # tile stays full-width so the psum accumulators
                            # initialize every column.
                            o, e = 0, IB
                            if idx > 0:
                                if jt >= 4 * ib:  # diag tile
                                    o = (jt - 4 * ib) * P
                                if jt < 4 * ib - 4:  # window tile
                                    e = min(IB, jt * P + (WINDOW - 1) + P
                                            - ib * IB + 1)
                            w = e - o
                            sps = psS.tile([P, IB], F32, tag="psS",
                                           name=f"sps_{head}_{jt}")
                            for h in range(2):
                                nc.tensor.matmul(
                                    sps[:, o:e],
                                    lhsT=kT_sb[:, h, jt * P:(jt + 1) * P],
                                    rhs=qblk[:, head * 2 + h, o:e],
                                    start=h == 0, stop=h == 1)
                            s1 = sS.tile([P, IB], F32, tag="tanh")
                            nc.scalar.activation(out=s1[:, o:e], in_=sps[:, o:e],
                                                 func=AF.Tanh,
                                                 scale=1.0 / SOFT_CAP)
                            src = s1
                            if jt >= 4 * ib:  # causal edge
                                s2 = sS.tile([P, IB], F32, tag="sel")
                                nc.gpsimd.affine_select(
                                    out=s2[:, o:e], in_=s1[:, o:e],
                                    pattern=[[1, w]],
                                    base=ib * IB + o - jt * P,
                                    channel_multiplier=-1,
                                    compare_op=OP.is_ge, fill=NEG_BIG)
                                src = s2
                            elif jt < 4 * ib - 4:  # window edge
                                s2 = sS.tile([P, IB], F32, tag="sel")
                                nc.gpsimd.affine_select(
                                    out=s2[:, o:e], in_=s1[:, o:e],
                                    pattern=[[-1, w]],
                                    base=jt * P + (WINDOW - 1) - ib * IB - o,
                                    channel_multiplier=1,
                                    compare_op=OP.is_ge, fill=NEG_BIG)
                                src = s2
                            pt = pPT.tile([P, IB], F32R)
                            nc.scalar.activation(out=pt[:, o:e], in_=src[:, o:e],
                                                 func=AF.Exp,
                                                 scale=SOFT_CAP, bias=nbias)
                            for h in range(2):
                                nc.tensor.matmul(
                                    ops[h][:, o:e],
                                    lhsT=v_sb[:, jt, h * P:(h + 1) * P],
                                    rhs=pt[:, o:e], start=idx == 0,
                                    stop=idx == len(jts) - 1)
                            nc.tensor.matmul(
                                dps[:, o:e], lhsT=ones, rhs=pt[:, o:e],
                                start=idx == 0, stop=idx == len(jts) - 1)
                        rc = dR.tile([P, IB], F32)
                        nc.vector.reciprocal(out=rc, in_=dps)
                        for h in range(2):
                            nc.vector.tensor_tensor(
                                out=ot_t[:, head * 2 + h, :], in0=ops[h], in1=rc,
                                op=OP.mult)

                    # o_proj for this query block
                    for st in range(4):
                        for eg in range(2):
                            pso = [psOut.tile([P, IB], F32, tag="psDO", name=f"psOut_{st}_{_}") for _ in range(2)]
                            for ct in range(2 * GROUPS):
                                for e2 in range(2):
                                    e = eg * 2 + e2
                                    nc.tensor.matmul(
                                        pso[e2],
                                        lhsT=ot_t[:, ct, st * P:(st + 1) * P],
                                        rhs=wo_s[:, ct, e * IB:(e + 1) * IB],
                                        start=ct == 0, stop=ct == 2 * GROUPS - 1)
                            for e2 in range(2):
                                ob = osb.tile([P, IB], F32, tag="osb",
                                              name=f"osb_{st}_{e2}")
                                nc.vector.tensor_copy(out=ob, in_=pso[e2])
                                nc.sync.dma_start(
                                    out=outd.ap()[
                                        ib * IB + st * P: ib * IB + (st + 1) * P,
                                        (eg * 2 + e2) * IB:(eg * 2 + e2 + 1) * IB],
                                    in_=ob)

    nc.finalize()
    return nc


_NC = None


def _get_nc():
    global _NC
    if _NC is None:
        _NC = _build()
    return _NC


def _prep_inputs(x, positions, attn_mask, wq, wk, wv, wo, q_scale, k_scale):
    x = np.asarray(x, dtype=np.float32)
    positions = np.asarray(positions)
    wq = np.asarray(wq, dtype=np.float32)
    wk = np.asarray(wk, dtype=np.float32)
    wv = np.asarray(wv, dtype=np.float32)
    wo = np.asarray(wo, dtype=np.float32)
    q_scale = np.asarray(q_scale, dtype=np.float32)
    k_scale = np.asarray(k_scale, dtype=np.float32)

    qsc = np.ascontiguousarray(q_scale.reshape(2, P).T)
    ksc = np.ascontiguousarray(k_scale.reshape(2, P).T)

    in_maps = []
    for core in range(8):
        b, g = core // 4, core % 4
        pos = positions[b].astype(np.float64)
        freq = ROPE_BASE ** (2.0 * np.arange(HALF, dtype=np.float64) / HEAD_DIM)
        t = pos[None, :] / freq[:, None] / ROPE_SCALE
        in_maps.append({
            "xT": np.ascontiguousarray(x[b].T),
            "wq": np.ascontiguousarray(wq[:, g * 512:(g + 1) * 512]),
            "wk": np.ascontiguousarray(wk[:, g * 256:(g + 1) * 256]),
            "wv": np.ascontiguousarray(wv[:, g * 256:(g + 1) * 256]),
            "wo": np.ascontiguousarray(wo[g * 512:(g + 1) * 512, :]),
            "cosT": np.cos(t).astype(np.float32),
            "sinT": np.sin(t).astype(np.float32),
            "qsc": qsc,
            "ksc": ksc,
        })
    return in_maps


def _assemble(results):
    out = np.zeros((B, L, D), dtype=np.float64)
    for core in range(8):
        out[core // 4] += results[core]["out"].astype(np.float64)
    return out.astype(np.float32)


def kernel(**inputs) -> np.ndarray:
    nc = _get_nc()
    in_maps = _prep_inputs(**inputs)
    last_exc = None
    for _attempt in range(3):
        try:
            res = run_bass_kernel_spmd(nc, in_maps, core_ids=list(range(8)))
            return _assemble(res.results)
        except Exception as e:  # transient NRT_EXEC_UNIT_UNRECOVERABLE wedges
            last_exc = e
            import time
            time.sleep(5)
    raise last_exc


def kernel_traced(**inputs):
    """Like kernel() but profiles all 8 cores; returns (output, results)."""
    nc = _get_nc()
    in_maps = _prep_inputs(**inputs)
    res = run_bass_kernel_spmd(nc, in_maps, core_ids=list(range(8)),
                               trace=True, trace_cores=list(range(8)))
    return _assemble(res.results), res



# revision 11
# speedup vs baseline: 1.4007x; 1.4007x over previous
"""GQA sliding-window attention (RoPE + RMSNorm + tanh soft-cap) on 8 trn2 cores.

Sharding: data-parallel over batch (2) x tensor-parallel over KV heads (4).
Core c handles batch b = c//4 and kv-head g = c%4 (2 query heads each).
o_proj is row-parallel; the 4 partial [L, D] outputs per batch are summed on
the host during unsharding.

Single fused phase per 512-query block: QKV projection -> RMSNorm+rope ->
attention -> o_proj, software-pipelined so the PE never idles:
  k(n), v(n), q(n) projections | o_proj(n-1) | attention(n) | k(n+1) ...
Device layout:
  - all matmul operands fp16 (1 cyc/row at 2.4 GHz); psums f32.
  - x fed transposed (xT [D, L]); q/k produced transposed ([head_dim, L]),
    v natural ([L, head_dim]); scores computed transposed (S^T [key, query]).
  - q lives in SBUF for the whole kernel (no DRAM roundtrip).
  - RMSNorm partition-sums via ones-matmul (result arrives pre-broadcast);
    rsqrt = scalar Sqrt + DVE reciprocal_approx_fast. Scalar-engine act
    tables: only sqrt<->exp/tanh swaps, 2 loads per block, placed in the
    o_proj window so they never stall the attention pipeline.
  - mask (causal + sliding window) applied post-exp with fill=0 on fp16
    via gpsimd.affine_select, only on tiles straddling a mask boundary.
  - softmax denominators from a ones-matmul accumulated alongside PV.
"""

import numpy as np
import ml_dtypes

import concourse.bass as bass
import concourse.bacc as bacc
import concourse.tile as tile
from concourse import mybir
from concourse.bass_utils import run_bass_kernel_spmd

F32 = mybir.dt.float32
F16 = mybir.dt.float16
BF16 = mybir.dt.bfloat16   # for softmax probs + v: exp(50*tanh-50) spans
                           # e^+-50, far beyond fp16 exponent range
AF = mybir.ActivationFunctionType
OP = mybir.AluOpType

B, L, D = 2, 2048, 2048
NUM_HEADS, NUM_KV_HEADS, HEAD_DIM = 8, 4, 256
GROUPS = NUM_HEADS // NUM_KV_HEADS  # 2
ROPE_BASE, ROPE_SCALE = 10000.0, 1.0
WINDOW = 1024
SOFT_CAP = 50.0
EPS = 1e-6
P = 128
IB = 512                      # query block
NB = L // IB                  # 4 query blocks
DT = D // P                   # 16 contraction tiles
HALF = HEAD_DIM // 2          # 128 (rope half == one partition tile)
NEG_BIG = -1e38


def _jts(ib):
    """key tiles (of 128) overlapping the causal sliding window of query
    block ib, diagonal-start tile first (it is always full width, so it
    starts the psum accumulation; every other tile gets trimmed). Tiles
    needing a mask op are interleaved with interior tiles so the gpsimd
    affine_selects never burst ahead of the PE."""
    i0 = ib * IB
    lo = max(0, (i0 - (WINDOW - 1)) // P)
    hi = (i0 + IB - 1) // P
    first = 4 * ib
    rest = [t for t in range(lo, hi + 1) if t != first]
    sel = [t for t in rest if t >= 4 * ib or t < 4 * ib - 4]
    nosel = [t for t in rest if not (t >= 4 * ib or t < 4 * ib - 4)]
    mixed = []
    while sel or nosel:
        if sel:
            mixed.append(sel.pop(0))
        if nosel:
            mixed.append(nosel.pop(0))
    return [first] + mixed


def _build():
    nc = bacc.Bacc("TRN2", target_bir_lowering=False, debug=False, num_devices=8)

    xT = nc.dram_tensor("xT", [D, L], F16, kind="ExternalInput")
    wq = nc.dram_tensor("wq", [D, GROUPS * HEAD_DIM], F16, kind="ExternalInput")
    wk = nc.dram_tensor("wk", [D, HEAD_DIM], F16, kind="ExternalInput")
    wv = nc.dram_tensor("wv", [D, HEAD_DIM], F16, kind="ExternalInput")
    wo = nc.dram_tensor("wo", [GROUPS * HEAD_DIM, D], F16, kind="ExternalInput")
    cosT = nc.dram_tensor("cosT", [HALF, L], F32, kind="ExternalInput")
    sinT = nc.dram_tensor("sinT", [HALF, L], F32, kind="ExternalInput")
    qsc = nc.dram_tensor("qsc", [P, 2], F32, kind="ExternalInput")
    ksc = nc.dram_tensor("ksc", [P, 2], F32, kind="ExternalInput")
    outd = nc.dram_tensor("out", [L, D], F32, kind="ExternalOutput")

    with tile.TileContext(nc) as tc, \
         tc.tile_pool(name="konst", bufs=1) as konst, \
         tc.tile_pool(name="kv_sb", bufs=1) as kv_sb, \
         tc.tile_pool(name="wts", bufs=1) as wts, \
         tc.tile_pool(name="xA", bufs=8) as xA, \
         tc.tile_pool(name="csA", bufs=2) as csA, \
         tc.tile_pool(name="nrm", bufs=2) as nrm, \
         tc.tile_pool(name="att", bufs=4) as att, \
         tc.tile_pool(name="oT", bufs=2) as oT, \
         tc.tile_pool(name="osb", bufs=6) as osb, \
         tc.tile_pool(name="psProj", bufs=2, space="PSUM") as psProj, \
         tc.tile_pool(name="psS", bufs=2, space="PSUM") as psS, \
         tc.tile_pool(name="psAcc", bufs=4, space="PSUM") as psAcc:

        ones_f = konst.tile([P, P], F32)
        nc.vector.memset(ones_f, 1.0)
        ones = konst.tile([P, P], F16)
        nc.vector.tensor_copy(out=ones, in_=ones_f)
        onesb = konst.tile([P, P], BF16)
        nc.vector.tensor_copy(out=onesb, in_=ones_f)
        nbias = konst.tile([P, 1], F32)
        nc.vector.memset(nbias, -SOFT_CAP)
        epsb = konst.tile([P, 1], F32)
        nc.vector.memset(epsb, EPS)

        kT_sb = kv_sb.tile([P, 2, L], F16)         # k^T (roped), h-halves
        v_sb = kv_sb.tile([P, DT, HEAD_DIM], BF16)  # v natural, j-tiles
        q_sb = kv_sb.tile([P, 2 * GROUPS, L], F16)  # q^T (roped), per (head, half)

        # ---- weights: split loads so first matmuls start early ----
        wk_s = wts.tile([P, DT, HEAD_DIM], F16)
        wv_s = wts.tile([P, DT, HEAD_DIM], F16)
        wq_s = wts.tile([P, DT, GROUPS * HEAD_DIM], F16)
        wo_s = wts.tile([P, 2 * GROUPS, D], F16)
        qsc_s = wts.tile([P, 2], F32)
        ksc_s = wts.tile([P, 2], F32)

        def load_w(dst, src, lo_dt, hi_dt):
            nc.sync.dma_start(
                out=dst[:, lo_dt:hi_dt, :],
                in_=src.ap()[lo_dt * P:hi_dt * P, :].rearrange(
                    "(dt dk) c -> dk dt c", dk=P))

        x_tiles = [None] * NB
        cs_tiles = [None] * NB

        def load_x(n):
            nsl = slice(n * IB, (n + 1) * IB)
            xq = []
            for qd in range(4):
                xt_ = xA.tile([P, DT // 4, IB], F16, tag="xh",
                              name=f"xh_{n}_{qd}")
                nc.sync.dma_start(
                    out=xt_,
                    in_=xT.ap()[qd * D // 4:(qd + 1) * D // 4, nsl].rearrange(
                        "(do dk) n -> dk do n", dk=P))
                xq.append(xt_)
            x_tiles[n] = xq

        def load_cs(n):
            nsl = slice(n * IB, (n + 1) * IB)
            cosb = csA.tile([P, IB], F32, tag="cos", name=f"cos_{n}")
            nc.sync.dma_start(out=cosb, in_=cosT.ap()[:, nsl])
            sinb = csA.tile([P, IB], F32, tag="sin", name=f"sin_{n}")
            nc.sync.dma_start(out=sinb, in_=sinT.ap()[:, nsl])
            cs_tiles[n] = (cosb, sinb)

        load_w(wk_s, wk, 0, 8)
        load_x(0)
        load_w(wk_s, wk, 8, DT)
        load_cs(0)
        nc.sync.dma_start(out=ksc_s, in_=ksc.ap())
        nc.sync.dma_start(out=qsc_s, in_=qsc.ap())
        load_w(wv_s, wv, 0, 8)
        load_w(wv_s, wv, 8, DT)
        load_w(wq_s, wq, 0, 8)
        load_w(wq_s, wq, 8, DT)
        load_x(1)
        nc.sync.dma_start(
            out=wo_s[:, :, :D // 2],
            in_=wo.ap()[:, :D // 2].rearrange("(ct ck) e -> ck ct e", ck=P))
        nc.sync.dma_start(
            out=wo_s[:, :, D // 2:],
            in_=wo.ap()[:, D // 2:].rearrange("(ct ck) e -> ck ct e", ck=P))
        load_cs(1)
        load_cs(2)
        load_cs(3)

        def dtile(n, dt):
            return x_tiles[n][dt // 4][:, dt % 4, :]

        def norm_sq(cops, tagp):
            """Squares of the two psum-copy halves (DVE), bf16 out."""
            sqs = []
            for i, cop in enumerate(cops):
                sq = nrm.tile([P, IB], F16, tag="sq", name=f"sq{i}_{tagp}")
                nc.vector.tensor_tensor(out=sq, in0=cop, in1=cop, op=OP.mult)
                sqs.append(sq)
            return sqs

        def norm_ssps(sqs, tagp):
            """Partition-dim sum of squares via ones-matmul (PE); the result
            arrives broadcast across all partitions."""
            ssps = psS.tile([P, IB], F32, tag="ps", name=f"ssps_{tagp}")
            nc.tensor.matmul(ssps, lhsT=ones, rhs=sqs[0], start=True, stop=False)
            nc.tensor.matmul(ssps, lhsT=ones, rhs=sqs[1], start=False, stop=True)
            return ssps

        def norm_fin(ssps, cop0, cop1, sc, n, dst0, dst1, tagp):
            """sqrt (scalar) -> reciprocal_approx_fast (DVE) -> channel scale
            + rope rotate (DVE) -> bf16 dst."""
            cosb, sinb = cs_tiles[n]
            srt = nrm.tile([P, IB], F32, tag="srt", name=f"srt_{tagp}")
            nc.scalar.activation(out=srt, in_=ssps, func=AF.Sqrt,
                                 scale=1.0 / HEAD_DIM, bias=epsb)
            rs = nrm.tile([P, IB], F32, tag="rs", name=f"rs_{tagp}")
            nc.vector.reciprocal_approx_fast(out=rs, in_=srt)
            a0 = nrm.tile([P, IB], F32, tag="a0", name=f"a0_{tagp}")
            nc.vector.scalar_tensor_tensor(
                out=a0, in0=cop0, scalar=sc[:, 0:1], in1=rs,
                op0=OP.mult, op1=OP.mult)
            a1 = nrm.tile([P, IB], F32, tag="a1", name=f"a1_{tagp}")
            nc.vector.scalar_tensor_tensor(
                out=a1, in0=cop1, scalar=sc[:, 1:2], in1=rs,
                op0=OP.mult, op1=OP.mult)
            t0 = nrm.tile([P, IB], F32, tag="t0", name=f"t0_{tagp}")
            nc.vector.tensor_tensor(out=t0, in0=a0, in1=cosb, op=OP.mult)
            t1 = nrm.tile([P, IB], F32, tag="t1", name=f"t1_{tagp}")
            nc.vector.tensor_tensor(out=t1, in0=a1, in1=sinb, op=OP.mult)
            nc.vector.tensor_tensor(out=dst0, in0=t0, in1=t1, op=OP.subtract)
            t2 = nrm.tile([P, IB], F32, tag="t0", name=f"t2_{tagp}")
            nc.vector.tensor_tensor(out=t2, in0=a1, in1=cosb, op=OP.mult)
            t3 = nrm.tile([P, IB], F32, tag="t1", name=f"t3_{tagp}")
            nc.vector.tensor_tensor(out=t3, in0=a0, in1=sinb, op=OP.mult)
            nc.vector.tensor_tensor(out=dst1, in0=t2, in1=t3, op=OP.add)

        def proj_pair(n, w_s, c0, tagp):
            """Project two 128-column chunks [c0, c0+256) of w; copy psums to
            SBUF f32 (frees the psum slots fast) and return the copies."""
            cops = []
            for h in range(2):
                ct = c0 + h * P
                ps = psProj.tile([P, IB], F32, tag="proj", name=f"pp_{tagp}_{h}")
                for dt in range(DT):
                    nc.tensor.matmul(
                        ps, lhsT=w_s[:, dt, ct:ct + P],
                        rhs=dtile(n, dt), start=dt == 0, stop=dt == DT - 1)
                cop = nrm.tile([P, IB], F32, tag="cop", bufs=6,
                               name=f"cop_{tagp}_{h}")
                nc.vector.tensor_copy(out=cop, in_=ps)
                cops.append(cop)
            return cops

        def v_proj(n):
            for st in range(4):
                ps = psProj.tile([P, IB], F32, tag="proj", name=f"vp_{n}_{st}")
                for dt in range(DT):
                    nc.tensor.matmul(
                        ps[:, :HEAD_DIM],
                        lhsT=dtile(n, dt)[:, st * P:(st + 1) * P],
                        rhs=wv_s[:, dt, :], start=dt == 0, stop=dt == DT - 1)
                nc.scalar.activation(out=v_sb[:, n * 4 + st, :],
                                     in_=ps[:, :HEAD_DIM], func=AF.Copy)

        def attention(n):
            jts = _jts(n)
            for head in range(GROUPS):
                ops = [psAcc.tile([P, IB], F32, tag="acc",
                                  name=f"ops_{n}_{head}_{h}") for h in range(2)]
                dps = psAcc.tile([P, IB], F32, tag="acc", name=f"dps_{n}_{head}")
                for idx, jt in enumerate(jts):
                    # active column range [o, e) of this score tile (queries
                    # outside it are fully masked); the leading diagonal tile
                    # is always full width and starts the accumulators.
                    o = max(0, (jt - 4 * n) * P)
                    e = min(IB, jt * P + WINDOW + P - n * IB)
                    w = e - o
                    sps = psS.tile([P, IB], F32, tag="ps",
                                   name=f"sps_{n}_{head}_{jt}")
                    for h in range(2):
                        nc.tensor.matmul(
                            sps[:, o:e],
                            lhsT=kT_sb[:, h, jt * P:(jt + 1) * P],
                            rhs=q_sb[:, head * 2 + h, n * IB + o:n * IB + e],
                            start=h == 0, stop=h == 1)
                    s1 = att.tile([P, IB], F16, tag="tanh")
                    nc.scalar.activation(out=s1[:, o:e], in_=sps[:, o:e],
                                         func=AF.Tanh, scale=1.0 / SOFT_CAP)
                    pt = att.tile([P, IB], BF16, tag="pt")
                    nc.scalar.activation(out=pt[:, o:e], in_=s1[:, o:e],
                                         func=AF.Exp, scale=SOFT_CAP, bias=nbias)
                    src = pt
                    if jt >= 4 * n:  # causal edge
                        p2 = att.tile([P, IB], BF16, tag="sel")
                        nc.gpsimd.affine_select(
                            out=p2[:, o:e], in_=pt[:, o:e],
                            pattern=[[1, w]],
                            base=n * IB + o - jt * P,
                            channel_multiplier=-1,
                            compare_op=OP.is_ge, fill=0.0)
                        src = p2
                    elif jt < 4 * n - 4:  # window edge
                        p2 = att.tile([P, IB], BF16, tag="sel")
                        nc.gpsimd.affine_select(
                            out=p2[:, o:e], in_=pt[:, o:e],
                            pattern=[[-1, w]],
                            base=jt * P + (WINDOW - 1) - n * IB - o,
                            channel_multiplier=1,
                            compare_op=OP.is_ge, fill=0.0)
                        src = p2
                    for h in range(2):
                        nc.tensor.matmul(
                            ops[h][:, o:e],
                            lhsT=v_sb[:, jt, h * P:(h + 1) * P],
                            rhs=src[:, o:e], start=idx == 0,
                            stop=idx == len(jts) - 1)
                    nc.tensor.matmul(
                        dps[:, o:e], lhsT=onesb, rhs=src[:, o:e],
                        start=idx == 0, stop=idx == len(jts) - 1)
                rc = att.tile([P, IB], F32, tag="rc", bufs=2)
                nc.vector.reciprocal_approx_fast(out=rc, in_=dps)
                for h in range(2):
                    nc.vector.tensor_tensor(
                        out=ot_t[n % 2][:, head * 2 + h, :], in0=ops[h], in1=rc,
                        op=OP.mult)

        def o_proj(n, chunks):
            ot = ot_t[n % 2]
            for st, eg in chunks:
                pso = [psAcc.tile([P, IB], F32, tag="acc",
                                  name=f"pso_{n}_{st}_{eg}_{e2}")
                       for e2 in range(2)]
                for ct in range(2 * GROUPS):
                    for e2 in range(2):
                        e = eg * 2 + e2
                        nc.tensor.matmul(
                            pso[e2],
                            lhsT=ot[:, ct, st * P:(st + 1) * P],
                            rhs=wo_s[:, ct, e * IB:(e + 1) * IB],
                            start=ct == 0, stop=ct == 2 * GROUPS - 1)
                for e2 in range(2):
                    ob = osb.tile([P, IB], F32, tag="osb",
                                  name=f"osb_{n}_{st}_{e2}")
                    nc.scalar.activation(out=ob, in_=pso[e2], func=AF.Copy)
                    nc.sync.dma_start(
                        out=outd.ap()[
                            n * IB + st * P: n * IB + (st + 1) * P,
                            (eg * 2 + e2) * IB:(eg * 2 + e2 + 1) * IB],
                        in_=ob)

        ot_t = [oT.tile([P, 2 * GROUPS, IB], F16, tag="ot", name=f"ot_{i}")
                for i in range(2)]

        ALL_CHUNKS = [(st, eg) for st in range(4) for eg in range(2)]
        for n in range(NB):
            nsl = slice(n * IB, (n + 1) * IB)
            if n + 2 < NB:
                load_x(n + 2)
            # k projection + norm + rope (ssps deferred past v_proj so the
            # PE never waits on the squares)
            kc = proj_pair(n, wk_s, 0, f"k_{n}")
            sqk = norm_sq(kc, f"k_{n}")
            v_proj(n)
            ssk = norm_ssps(sqk, f"k_{n}")
            norm_fin(ssk, kc[0], kc[1], ksc_s, n,
                     kT_sb[:, 0, nsl], kT_sb[:, 1, nsl], f"k_{n}")
            # q projections + norm + rope (straight into SBUF q_sb); each
            # head's ssps lands after enough independent PE work to cover
            # the square/copy latency, with an o_proj chunk as filler.
            qc0 = proj_pair(n, wq_s, 0, f"q_{n}_0")
            sq0 = norm_sq(qc0, f"q_{n}_0")
            qc1 = proj_pair(n, wq_s, 2 * P, f"q_{n}_1")
            ss0 = norm_ssps(sq0, f"q_{n}_0")
            norm_fin(ss0, qc0[0], qc0[1], qsc_s, n,
                     q_sb[:, 0, nsl], q_sb[:, 1, nsl], f"q_{n}_0")
            sq1 = norm_sq(qc1, f"q_{n}_1")
            if n > 0:
                o_proj(n - 1, ALL_CHUNKS[:1])
            ss1 = norm_ssps(sq1, f"q_{n}_1")
            norm_fin(ss1, qc1[0], qc1[1], qsc_s, n,
                     q_sb[:, 2, nsl], q_sb[:, 3, nsl], f"q_{n}_1")
            if n > 0:
                o_proj(n - 1, ALL_CHUNKS[1:])
            attention(n)
        o_proj(NB - 1, ALL_CHUNKS)

    nc.finalize()
    return nc


_NC = None


def _get_nc():
    global _NC
    if _NC is None:
        _NC = _build()
    return _NC


def _prep_inputs(x, positions, attn_mask, wq, wk, wv, wo, q_scale, k_scale):
    f16 = np.float16
    x = np.asarray(x, dtype=np.float32)
    positions = np.asarray(positions)
    wq = np.asarray(wq, dtype=np.float32)
    wk = np.asarray(wk, dtype=np.float32)
    wv = np.asarray(wv, dtype=np.float32)
    wo = np.asarray(wo, dtype=np.float32)
    q_scale = np.asarray(q_scale, dtype=np.float32)
    k_scale = np.asarray(k_scale, dtype=np.float32)

    qsc = np.ascontiguousarray(q_scale.reshape(2, P).T)
    ksc = np.ascontiguousarray(k_scale.reshape(2, P).T)

    in_maps = []
    for core in range(8):
        b, g = core // 4, core % 4
        pos = positions[b].astype(np.float64)
        freq = ROPE_BASE ** (2.0 * np.arange(HALF, dtype=np.float64) / HEAD_DIM)
        t = pos[None, :] / freq[:, None] / ROPE_SCALE
        in_maps.append({
            "xT": np.ascontiguousarray(x[b].T).astype(f16),
            "wq": np.ascontiguousarray(wq[:, g * 512:(g + 1) * 512]).astype(f16),
            "wk": np.ascontiguousarray(wk[:, g * 256:(g + 1) * 256]).astype(f16),
            "wv": np.ascontiguousarray(wv[:, g * 256:(g + 1) * 256]).astype(f16),
            "wo": np.ascontiguousarray(wo[g * 512:(g + 1) * 512, :]).astype(f16),
            "cosT": np.cos(t).astype(np.float32),
            "sinT": np.sin(t).astype(np.float32),
            "qsc": qsc,
            "ksc": ksc,
        })
    return in_maps


def _assemble(results):
    out = np.zeros((B, L, D), dtype=np.float64)
    for core in range(8):
        out[core // 4] += results[core]["out"].astype(np.float64)
    return out.astype(np.float32)


def kernel(**inputs) -> np.ndarray:
    nc = _get_nc()
    in_maps = _prep_inputs(**inputs)
    last_exc = None
    for _attempt in range(3):
        try:
            res = run_bass_kernel_spmd(nc, in_maps, core_ids=list(range(8)))
            return _assemble(res.results)
        except Exception as e:  # transient NRT_EXEC_UNIT_UNRECOVERABLE wedges
            last_exc = e
            import time
            time.sleep(5)
    raise last_exc


def kernel_traced(**inputs):
    """Like kernel() but profiles all 8 cores; returns (output, results)."""
    nc = _get_nc()
    in_maps = _prep_inputs(**inputs)
    res = run_bass_kernel_spmd(nc, in_maps, core_ids=list(range(8)),
                               trace=True, trace_cores=list(range(8)))
    return _assemble(res.results), res
